# revision 26
# baseline (speedup 1.0000x reference)
"""Trainium2 Bass kernel for nn_CrossAttn_65214783422649.

Key algebraic reduction: softmax over R followed by mean over R is identically
1/R, so the whole attention branch (Wq, Wk, energy, softmax) cancels out of
the output:

    sims[i, c] = (a_c + b_i) . cs_c / (||a_c + b_i|| ||cs_c||)
      a_c = (gamma/R) * sum_t mask * leaky(cap_c @ Wvt.T + bvt)
      b_i = mean_r leaky(img_i @ Wvi.T + bvi)
      cs_c = masked-sum_t cap_c        (the 1/len cancels under l2norm)

Layouts are prepared host-side: weights / images / captions are pre-transposed
into [d-contract-on-partition] tiles and cast to bf16, so the kernel issues no
on-chip layout transposes or dtype-cast copies for the big matmuls.  Host
inputs are consolidated into a few large DMAs (the Sync engine issues DMAs
serially at ~0.6us each, so DMA count is a startup cost).  The image phase
runs k-outer over it-pairs so the PE starts as soon as the first interleaved
[Wvi_k | img_k] chunk lands.  Valid caption tokens are host-packed (ragged
lens) with a balanced (LPT) caption -> core assignment.  An on-chip AllGather
shares the per-core b shards (+ squared norms) before the similarity
assembly, which runs in bf16 (f32 PSUM accumulation).
"""

import numpy as np
import ml_dtypes

import concourse.bass as bass
import concourse.mybir as mybir
import concourse.tile as tile
from concourse import bacc
from concourse.bass import ds, ts
from concourse.bass_utils import run_bass_kernel_spmd

F32 = mybir.dt.float32
BF16 = mybir.dt.bfloat16
AF = mybir.ActivationFunctionType
NPBF = ml_dtypes.bfloat16

N_CORES = 8
B_I, B_C, R, T, D = 128, 128, 36, 64, 1024
C_SH = B_C // N_CORES          # 16 captions per core
I_SH = B_I // N_CORES          # 16 images per core
IMG_TOK = I_SH * R             # 576 image tokens per core
IMG_PAD = 640                  # padded to 5 * 128
KT = D // 128                  # 8 contraction tiles
IT = IMG_PAD // 128            # 5 image token tiles
NEG_SLOPE = 0.1
AGW = 1025                     # AllGather row: b row (1024) + |b|^2
IMGW = D + IMG_PAD             # per-k interleaved [Wvi_k | img_k] chunk

_CACHE: dict = {}


def _build(CT: int, has_bias: bool):
    """CT = number of 128-token caption tiles after host packing."""
    CAP_TOK = CT * 128
    # smalls blob layout (bf16): om | om_b | identity | ones
    OM_OFF = 0
    OMB_OFF = OM_OFF + CT * C_SH
    IDB_OFF = OMB_OFF + IT * I_SH
    ONES_OFF = IDB_OFF + 128
    SMW = ONES_OFF + 17
    # capw blob layout (bf16): wvtT | capT
    CAPT_OFF = KT * D
    CAPW = CAPT_OFF + KT * CAP_TOK

    nc = bacc.Bacc("TRN2", target_bir_lowering=False, debug=False,
                   num_devices=N_CORES)

    smalls_d = nc.dram_tensor("smalls", [128, SMW], BF16, kind="ExternalInput")
    imgw_d = nc.dram_tensor("imgw", [128, KT, IMGW], BF16,
                            kind="ExternalInput")
    capw_d = nc.dram_tensor("capw", [128, CAPW], BF16, kind="ExternalInput")
    capn_d = nc.dram_tensor("capn", [128, CT, D], BF16, kind="ExternalInput")
    gam_d = nc.dram_tensor("gam16", [C_SH, 1], F32, kind="ExternalInput")
    if has_bias:
        bfi_d = nc.dram_tensor("bias_vi", [128, D], F32, kind="ExternalInput")
        bft_d = nc.dram_tensor("bias_vt", [128, D], F32, kind="ExternalInput")
    sims_d = nc.dram_tensor("sims", [C_SH, B_I], F32, kind="ExternalOutput")

    with tile.TileContext(nc) as tc:
        with (
            tc.tile_pool(name="const", bufs=1) as const,
            tc.tile_pool(name="xt", bufs=1) as xtp,
            tc.tile_pool(name="vtx", bufs=3) as vtxp,
            tc.tile_pool(name="gpool", bufs=1) as gp,
            tc.tile_pool(name="small", bufs=1) as sp,
            tc.tile_pool(name="ps_mm", bufs=6, space="PSUM") as ps_mm,
            tc.tile_pool(name="ps_acc", bufs=1, space="PSUM") as ps_acc,
            tc.tile_pool(name="dram", bufs=1, space="DRAM") as dram,
        ):
            # ---- input DMAs, ordered by need time (few and large: the Sync
            # engine issues DMAs serially at ~0.6us each)
            smalls = const.tile([128, SMW], BF16, tag="smalls")
            nc.sync.dma_start(out=smalls[:], in_=smalls_d[:, :])
            imgw = []
            for k in range(KT):
                xk = xtp.tile([128, IMGW], BF16, tag=f"imgw{k}")
                nc.sync.dma_start(out=xk[:], in_=imgw_d[:, k, :])
                imgw.append(xk)
            capw = xtp.tile([128, CAPW], BF16, tag="capw")
            nc.sync.dma_start(out=capw[:], in_=capw_d[:, :])
            capn = xtp.tile([128, CT, D], BF16, tag="capn")
            nc.sync.dma_start(out=capn[:], in_=capn_d[:, :, :])
            gam16 = const.tile([C_SH, 1], F32, tag="gam16")
            nc.sync.dma_start(out=gam16[:], in_=gam_d[:, :])
            if has_bias:
                bias_vi = const.tile([128, D], F32, tag="bias_vi")
                nc.sync.dma_start(out=bias_vi[:], in_=bfi_d[:, :])
                bias_vt = const.tile([128, D], F32, tag="bias_vt")
                nc.sync.dma_start(out=bias_vt[:], in_=bft_d[:, :])

            # ---- HAM warm-up: keep the PE busy with throwaway matmuls while
            # the first imgw chunks stream in, so the clock gate is at 8/8
            # (2.4 GHz) when the real matmuls start (saves the ~3.4us
            # half-rate ramp).  The dummies finish before the first chunk
            # lands, so they never delay real work.
            warm = const.tile([128, 128], BF16, tag="warm")
            nc.vector.memset(warm[:], 0.0)
            ps_w = ps_mm.tile([128, 128], F32, tag="mm", name="ps_warm")
            for _ in range(28):
                nc.tensor.matmul(ps_w[:], warm[:], warm[:],
                                 start=True, stop=True)

            def om_sl(ct):
                return smalls[:, ds(OM_OFF + ct * C_SH, C_SH)]

            def omb_sl(it):
                return smalls[:, ds(OMB_OFF + it * I_SH, I_SH)]

            identb = smalls[:, ds(IDB_OFF, 128)]
            identb16 = smalls[0:C_SH, ds(IDB_OFF, C_SH)]
            ones_col = smalls[:, ds(ONES_OFF, 1)]
            ones_row16 = smalls[0:1, ds(ONES_OFF + 1, C_SH)]

            # ---- img matmul phase (k-outer over it-groups so the PE starts
            # on the first arrived chunk; 6 PSUM banks let each arriving
            # chunk feed 6 matmuls, keeping the PE saturated while the
            # remaining chunks stream in) -> b shard, single AllGather
            ps_b = [ps_acc.tile([I_SH, 512], F32, tag=f"acc{dh}",
                                name=f"ps_b{dh}")[:] for dh in range(2)]
            for pi, pair in enumerate(((0, 1, 2), (3,), (4,))):
                pms = {}
                for it in pair:
                    for dh in range(2):
                        pms[(it, dh)] = ps_mm.tile(
                            [128, 512], F32, tag="mm", name=f"pmi{it}{dh}")
                for k in range(KT):
                    for it in pair:
                        for dh in range(2):
                            nc.tensor.matmul(
                                pms[(it, dh)][:],
                                imgw[k][:, ds(D + it * 128, 128)],
                                imgw[k][:, ds(dh * 512, 512)],
                                start=(k == 0), stop=(k == KT - 1))
                for it in pair:
                    for dh in range(2):
                        pm = pms[(it, dh)]
                        if has_bias:
                            nc.vector.tensor_add(
                                pm[:], pm[:], bias_vi[:, ds(dh * 512, 512)])
                        vimg = vtxp.tile([128, 512], BF16, tag="vtx",
                                         name=f"vimg{it}{dh}")
                        nc.scalar.activation(vimg[:], pm[:], AF.Prelu,
                                             alpha=NEG_SLOPE)
                        nc.tensor.matmul(ps_b[dh], omb_sl(it), vimg[:],
                                         start=(it == 0), stop=(it == IT - 1))
            bnat = sp.tile([I_SH, D], BF16, tag="bnat")
            for dh in range(2):
                nc.vector.tensor_scalar_mul(bnat[:, ds(dh * 512, 512)],
                                            ps_b[dh], 1.0 / R)
            ag_in = dram.tile([I_SH, D], BF16, tag="ag_in")
            ag_out = dram.tile([B_I, D], BF16, addr_space="Shared",
                               tag="ag_out")
            nc.sync.dma_start(out=ag_in[:], in_=bnat[:])
            nc.gpsimd.collective_compute(
                "AllGather",
                mybir.AluOpType.bypass,
                replica_groups=[list(range(N_CORES))],
                ins=[ag_in[:].opt()],
                outs=[ag_out[:].opt()],
            )

            # ---- cap matmul phase -> a, capsum (overlaps the AllGather)
            ps_a = [ps_acc.tile([C_SH, 512], F32, tag=f"acc{dh}",
                                name=f"ps_a{dh}")[:] for dh in range(2)]
            for ct in range(CT):
                for dh in range(2):
                    pm = ps_mm.tile([128, 512], F32, tag="mm",
                                    name=f"pmc{ct}{dh}")
                    for k in range(KT):
                        nc.tensor.matmul(
                            pm[:],
                            capw[:, ds(CAPT_OFF + k * CAP_TOK + ct * 128,
                                       128)],
                            capw[:, ds(k * D + dh * 512, 512)],
                            start=(k == 0), stop=(k == KT - 1))
                    if has_bias:
                        nc.vector.tensor_add(pm[:], pm[:],
                                             bias_vt[:, ds(dh * 512, 512)])
                    vtxt = vtxp.tile([128, 512], BF16, tag="vtx",
                                     name=f"vtxt{ct}{dh}")
                    nc.scalar.activation(vtxt[:], pm[:], AF.Prelu,
                                         alpha=NEG_SLOPE)
                    nc.tensor.matmul(ps_a[dh], om_sl(ct), vtxt[:],
                                     start=(ct == 0), stop=(ct == CT - 1))
            a_s = sp.tile([C_SH, D], BF16, tag="a_s")
            for dh in range(2):
                nc.scalar.activation(a_s[:, ds(dh * 512, 512)], ps_a[dh],
                                     AF.Identity, scale=gam16[:])
            # capsum reuses the freed acc banks (runs under the AG wait)
            ps_cm = [ps_acc.tile([C_SH, 512], F32, tag=f"acc{dh}",
                                 name=f"ps_cm{dh}")[:] for dh in range(2)]
            for ct in range(CT):
                for dh in range(2):
                    nc.tensor.matmul(ps_cm[dh], om_sl(ct),
                                     capn[:, ct, ds(dh * 512, 512)],
                                     start=(ct == 0), stop=(ct == CT - 1))
            cs_s = sp.tile([C_SH, D], BF16, tag="cs_s")
            for dh in range(2):
                nc.scalar.activation(cs_s[:, ds(dh * 512, 512)], ps_cm[dh],
                                     AF.Copy)

            # ---- similarity assembly (bf16 operands, f32 PSUM accum)
            aT = gp.tile([128, KT, C_SH], BF16, tag="aT")
            csT = gp.tile([128, KT, C_SH], BF16, tag="csT")
            for src, dst, nm in ((a_s, aT, "a"), (cs_s, csT, "c")):
                for k in range(KT):
                    pst = ps_mm.tile([128, C_SH], BF16, tag="mm",
                                     name=f"pq{nm}{k}")
                    nc.tensor.transpose(pst[:], src[:, ts(k, 128)], identb16)
                    nc.vector.tensor_copy(dst[:, k, :], pst[:])

            # pack columns at 0/32/64 so output rows land on legal partitions
            pack = gp.tile([128, 80], BF16, tag="pack")
            ps_sc = ps_acc.tile([80, 1], F32, tag="acc0")
            for k in range(KT):
                nc.vector.tensor_mul(pack[:, 0:C_SH], aT[:, k, :], csT[:, k, :])
                nc.vector.tensor_mul(pack[:, 32:32 + C_SH], aT[:, k, :],
                                     aT[:, k, :])
                nc.vector.tensor_mul(pack[:, 64:64 + C_SH], csT[:, k, :],
                                     csT[:, k, :])
                nc.tensor.matmul(ps_sc[:], pack[:], ones_col,
                                 start=(k == 0), stop=(k == KT - 1))
            sc_s = sp.tile([80, 1], F32, tag="sc_s")
            nc.vector.tensor_copy(sc_s[:], ps_sc[:])

            sqq = sp.tile([C_SH, 1], F32, tag="sqq")
            nc.scalar.activation(sqq[:], sc_s[64:64 + C_SH, :], AF.Sqrt)
            shat = sp.tile([C_SH, 1], F32, tag="shat")
            nc.vector.reciprocal(shat[:], sqq[:])

            # bT via DMA-transpose straight out of the AllGather buffers.
            # Issues alternate between the two HWDGE queues (Sync, Scalar) so
            # the serial ~0.6us per-DMA issue cost is halved; the first half
            # (dh=0) transfers while the second AllGather is still running.
            bT = gp.tile([128, KT, B_I], BF16, tag="bT")
            bsqT = gp.tile([128, KT, B_I], BF16, tag="bsqT")
            ps_nb = ps_mm.tile([1, B_I], F32, tag="mm", name="ps_nb")
            ps_g1 = ps_acc.tile([C_SH, B_I], F32, tag="acc1")
            ps_g2 = ps_mm.tile([C_SH, B_I], F32, tag="mm", name="ps_g2")
            nc.sync.dma_start(out=bT[:, :, :], in_=ag_out[:, :],
                              transpose=True)
            nc.vector.tensor_mul(bsqT[:], bT[:], bT[:])
            for k in range(KT):
                nc.tensor.matmul(ps_nb[:], ones_col, bsqT[:, k, :],
                                 start=(k == 0), stop=(k == KT - 1))
                nc.tensor.matmul(ps_g1[:], aT[:, k, :], bT[:, k, :],
                                 start=(k == 0), stop=False)
                nc.tensor.matmul(ps_g2[:], csT[:, k, :], bT[:, k, :],
                                 start=(k == 0), stop=(k == KT - 1))
            nb_row = sp.tile([1, B_I], BF16, tag="nb_row")
            nc.vector.tensor_scalar_mul(nb_row[:], ps_nb[:], 0.5)
            nc.tensor.matmul(ps_g1[:], ones_row16, nb_row[:, :],
                             start=False, stop=True)
            den = sp.tile([C_SH, B_I], F32, tag="den")
            nc.scalar.activation(den[:], ps_g1[:], AF.Sqrt, scale=2.0,
                                 bias=sc_s[32:32 + C_SH, :])
            num = sp.tile([C_SH, B_I], F32, tag="num")
            nc.vector.tensor_scalar(
                out=num[:], in0=ps_g2[:], scalar1=sc_s[0:C_SH, :],
                scalar2=shat[:], op0=mybir.AluOpType.add,
                op1=mybir.AluOpType.mult)
            rden = sp.tile([C_SH, B_I], F32, tag="rden")
            nc.vector.reciprocal(rden[:], den[:])
            sims_s = sp.tile([C_SH, B_I], F32, tag="sims_s")
            nc.vector.tensor_mul(sims_s[:], num[:], rden[:])
            nc.sync.dma_start(out=sims_d[:, :], in_=sims_s[:])

    nc.compile()
    return nc


def _get_nc(CT: int, has_bias: bool):
    key = (CT, has_bias)
    if key not in _CACHE:
        _CACHE[key] = _build(CT, has_bias)
    return _CACHE[key]


def _t_tiles(x):
    """[tok, D] f32 -> [128, KT, tok] bf16 with d-contract on partitions."""
    tok = x.shape[0]
    return np.ascontiguousarray(
        x.T.reshape(KT, 128, tok).transpose(1, 0, 2)).astype(NPBF)


def _host_prep(inputs):
    cap_embed = np.asarray(inputs["cap_embed"], dtype=np.float32)
    img_embed = np.asarray(inputs["img_embed"], dtype=np.float32)
    lens = np.asarray(inputs["lens"]).astype(np.int64)
    wvt = np.asarray(inputs["Wvt"], dtype=np.float32)
    wvi = np.asarray(inputs["Wvi"], dtype=np.float32)
    bvt = np.asarray(inputs["bvt"], dtype=np.float32)
    bvi = np.asarray(inputs["bvi"], dtype=np.float32)
    has_bias = bool(np.any(bvt) or np.any(bvi))
    gamma = float(np.asarray(inputs["gamma_img"]).reshape(-1)[0])

    # balanced (LPT) caption -> core assignment; exactly C_SH slots per core
    order = np.argsort(-lens, kind="stable")
    loads = np.zeros(N_CORES, np.int64)
    counts = np.zeros(N_CORES, np.int64)
    cap_ids = [[] for _ in range(N_CORES)]
    for c in order:
        elig = [m for m in range(N_CORES) if counts[m] < C_SH]
        m = min(elig, key=lambda mm: (loads[mm], mm))
        cap_ids[m].append(int(c))
        loads[m] += int(lens[c])
        counts[m] += 1
    CT = max(1, -(-int(loads.max()) // 128))
    CAP_TOK = CT * 128

    # weight tiles: [p, k, e] = W[e, k*128+p]
    wviT = np.ascontiguousarray(
        wvi.T.reshape(KT, 128, D).transpose(1, 0, 2)).astype(NPBF)
    wvtT = np.ascontiguousarray(
        wvt.T.reshape(KT, 128, D).transpose(1, 0, 2)).astype(NPBF)

    om_b = np.zeros((IMG_PAD, I_SH), np.float32)
    om_b[:IMG_TOK] = np.repeat(np.eye(I_SH, dtype=np.float32), R, axis=0)
    om_b_t = om_b.reshape(IT, 128, I_SH).transpose(1, 0, 2).reshape(128, -1)
    ident = np.eye(128, dtype=np.float32)
    ones = np.ones((128, 17), np.float32)
    gam16 = np.full((C_SH, 1), gamma / R, np.float32)
    bias_vi = np.ascontiguousarray(np.repeat(bvi.reshape(1, D), 128, axis=0))
    bias_vt = np.ascontiguousarray(np.repeat(bvt.reshape(1, D), 128, axis=0))

    in_maps = []
    for m in range(N_CORES):
        ids = cap_ids[m]
        im = slice(m * I_SH, (m + 1) * I_SH)
        cap = np.zeros((CAP_TOK, D), np.float32)
        om = np.zeros((CAP_TOK, C_SH), np.float32)
        pos = 0
        for slot, c in enumerate(ids):
            n = int(lens[c])
            cap[pos:pos + n] = cap_embed[c, :n]
            om[pos:pos + n, slot] = 1.0
            pos += n
        img = np.zeros((IMG_PAD, D), np.float32)
        img[:IMG_TOK] = img_embed[im].reshape(IMG_TOK, D)
        om_t = om.reshape(CT, 128, C_SH).transpose(1, 0, 2).reshape(128, -1)
        smalls = np.concatenate([om_t, om_b_t, ident, ones],
                                axis=1).astype(NPBF)
        imgw = np.concatenate(
            [wviT, _t_tiles(img)], axis=2)              # [128, KT, D+IMG_PAD]
        capw = np.concatenate(
            [wvtT.reshape(128, KT * D),
             _t_tiles(cap).reshape(128, KT * CAP_TOK)], axis=1)
        in_map = {
            "smalls": np.ascontiguousarray(smalls),
            "imgw": np.ascontiguousarray(imgw),
            "capw": np.ascontiguousarray(capw),
            "capn": np.ascontiguousarray(
                cap.reshape(CT, 128, D).transpose(1, 0, 2)).astype(NPBF),
            "gam16": gam16,
        }
        if has_bias:
            in_map["bias_vi"] = bias_vi
            in_map["bias_vt"] = bias_vt
        in_maps.append(in_map)
    return in_maps, CT, has_bias, cap_ids


def _unshard(res, cap_ids):
    sims = np.empty((B_I, B_C), np.float32)
    for m in range(N_CORES):
        sims[:, cap_ids[m]] = res.results[m]["sims"].T
    return sims


def kernel(**inputs) -> np.ndarray:
    in_maps, CT, has_bias, cap_ids = _host_prep(inputs)
    nc = _get_nc(CT, has_bias)
    res = run_bass_kernel_spmd(nc, in_maps, core_ids=list(range(N_CORES)))
    return _unshard(res, cap_ids)


def run_traced(**inputs):
    """For test.py: same as kernel() but with NTFF tracing enabled."""
    in_maps, CT, has_bias, cap_ids = _host_prep(inputs)
    nc = _get_nc(CT, has_bias)
    res = run_bass_kernel_spmd(nc, in_maps, core_ids=list(range(N_CORES)),
                               trace=True)
    return _unshard(res, cap_ids), res
